# revision 10
# baseline (speedup 1.0000x reference)
"""Causal self-attention (B=4, T=2048, C=1024, 16 heads, interleaved RoPE)
on 8 trn2 NeuronCores.

Sharding: tensor-parallel over heads -- core c owns heads {2c, 2c+1} (128
channels). Each core:
  - loads full x, transposes it on the PE to x^T tiles (fp32 exact),
  - projects Q^T/K^T/V^T [128=2*64, T] per batch in f32r (full-rate matmuls),
  - applies interleaved RoPE as: rope(p) = p*cos + Pswap @ (p*sin_signed),
    where Pswap is the even/odd swap permutation done as one PE matmul,
  - scores S^T[kv,q] = K Q^T per head; exp on ACT (scale=1/8, no max-sub
    needed: scores are ~N(0,1)); causal via column sub-ranges + triangular
    multiplicative mask (saves the masked work instead of computing it),
  - y^T = V_aug^T @ P^T where V_aug carries a ones column, fusing softmax
    row-sums into the PV matmul; normalize via reciprocal + gpsimd
    partition_broadcast,
  - partial output projection y^T_c @ wo_c -> [B, T, C] partial.
Host sums the 8 partials (the all-reduce of the hinted TP scheme).

Self-contained: hardcoded shapes, no reads of /root/problem/*.
"""
import math
import os

import numpy as np

import concourse.bacc as bacc
import concourse.mybir as mybir
import concourse.tile as tile
from concourse.bass_utils import run_bass_kernel_spmd
from concourse.masks import make_identity, make_upper_triangular

B, T, C = 4, 2048, 1024
NH, D = 16, 64
NCORES = 8
HL = NH // NCORES  # heads per core
HD = HL * D  # per-core head channels = 128
QTILE = 512
KB = T // 128  # kv blocks per batch = 16
NJ = T // QTILE  # q tiles per batch = 4
CB = C // 128  # channel blocks = 8
F32 = mybir.dt.float32
F32R = mybir.dt.float32r
EXP = mybir.ActivationFunctionType.Exp

_CACHE = {}


def build(num_batches=B):
    ablate = set(os.environ.get("ABLATE", "").split(",")) if os.environ.get("ABLATE") else set()
    nc = bacc.Bacc(None, target_bir_lowering=False)
    x_d = nc.declare_dram_parameter("x", [B, T, C], F32, isOutput=False)
    wq_d = nc.declare_dram_parameter("wqt", [C, HD], F32, isOutput=False)
    wk_d = nc.declare_dram_parameter("wkt", [C, HD], F32, isOutput=False)
    wv_d = nc.declare_dram_parameter("wvt", [C, HD], F32, isOutput=False)
    wo_d = nc.declare_dram_parameter("wot", [HD, C], F32, isOutput=False)
    cos_d = nc.declare_dram_parameter("cosb", [HD, T], F32, isOutput=False)
    sin_d = nc.declare_dram_parameter("sinb", [HD, T], F32, isOutput=False)
    psw_d = nc.declare_dram_parameter("pswap", [128, 128], F32, isOutput=False)
    out_d = nc.declare_dram_parameter("out", [B, T, C], F32, isOutput=True)

    with tile.TileContext(nc) as tc:
        with (
            tc.tile_pool(name="const", bufs=1) as const,
            tc.tile_pool(name="wpool", bufs=1) as wpool,
            tc.tile_pool(name="xsb", bufs=2) as xsb,
            tc.tile_pool(name="xtp", bufs=1) as xtp,
            tc.tile_pool(name="qkp", bufs=2) as qkp,
            tc.tile_pool(name="vtp", bufs=2) as vtp,
            tc.tile_pool(name="vap", bufs=2) as vap,
            tc.tile_pool(name="ytp", bufs=2) as ytp,
            tc.tile_pool(name="ptp", bufs=4) as ptp,
            tc.tile_pool(name="npool", bufs=2) as npool,
            tc.tile_pool(name="opool", bufs=3) as opool,
            tc.tile_pool(name="ps", bufs=2, space="PSUM") as ps,
        ):
            # ---- constants ----
            ident = const.tile([128, 128], F32)
            make_identity(nc, ident)
            tri_f = const.tile([128, 128], F32)
            make_upper_triangular(nc, tri_f, val=1.0, diag=True)  # 1 if i<=j
            tri = const.tile([128, 128], F32R)
            nc.vector.tensor_copy(tri[:], tri_f[:])
            ones_f = const.tile([128, 1], F32)
            nc.gpsimd.memset(ones_f[:], 1.0)
            zeros = const.tile([128, 128], F32R)
            nc.gpsimd.memset(zeros.bitcast(F32)[:], 0.0)
            cos_t = const.tile([HD, T], F32)
            sin_t = const.tile([HD, T], F32)
            nc.sync.dma_start(out=cos_t[:], in_=cos_d[:])
            nc.sync.dma_start(out=sin_t[:], in_=sin_d[:])

            # ---- weights -> f32r ----
            with tc.tile_pool(name="wstage", bufs=2) as wstage:
                def load_kxm(dram, name):
                    stg = wstage.tile([128, CB, HD], F32, name=f"{name}_f", tag="wst")
                    nc.sync.dma_start(
                        out=stg[:], in_=dram.ap().rearrange("(cb p) m -> p cb m", p=128)
                    )
                    wr = wpool.tile([128, CB, HD], F32R, name=f"{name}_r")
                    nc.vector.tensor_copy(wr[:], stg[:])
                    return wr

                wq_r = load_kxm(wq_d, "wq")
                wk_r = load_kxm(wk_d, "wk")
                wv_r = load_kxm(wv_d, "wv")
                wo_f = wstage.tile([HD, C], F32, tag="wst2")
                nc.sync.dma_start(out=wo_f[:], in_=wo_d[:])
                wo_r = wpool.tile([HD, C], F32R)
                nc.vector.tensor_copy(wo_r[:], wo_f[:])
                psw_f = wstage.tile([128, 128], F32, tag="wst3")
                nc.sync.dma_start(out=psw_f[:], in_=psw_d[:])
                psw_r = wpool.tile([128, 128], F32R)
                nc.vector.tensor_copy(psw_r[:], psw_f[:])

            for b in range(num_batches):
                # ============== stage 1: x^T, projections, rope ==============
                qt_b = qkp.tile([HD, T], F32R, name="qt_b", tag="qt")
                kt_b = qkp.tile([HD, T], F32R, name="kt_b", tag="kt")
                va_a = vap.tile([128, KB, D + 1], F32R, name="va_a", tag="va_a")
                va_b = vap.tile([128, KB, D + 1], F32R, name="va_b", tag="va_b")
                yt_b = ytp.tile([HD, T], F32R, name="yt_b")
                for tt in range(NJ):
                    t0 = tt * QTILE
                    x_sb = xsb.tile([128, 4, C], F32, name="x_sb")
                    nc.sync.dma_start(
                        out=x_sb[:],
                        in_=x_d.ap()[b, t0 : t0 + QTILE, :].rearrange(
                            "(ts p) c -> p ts c", p=128
                        ),
                    )
                    xt_t = xtp.tile([128, CB, QTILE], F32R, name="xt_t")
                    for cb in range(CB):
                        xt_ps = ps.tile([128, QTILE], F32, name="xt_ps", tag="tr")
                        for ts in range(4):
                            nc.tensor.transpose(
                                xt_ps[:, ts * 128 : (ts + 1) * 128],
                                x_sb[:, ts, cb * 128 : (cb + 1) * 128],
                                ident[:],
                            )
                        nc.vector.tensor_copy(xt_t[:, cb, :], xt_ps[:])

                    # q/k projections with fused rope
                    for wname, wr, dst in (("q", wq_r, qt_b), ("k", wk_r, kt_b)):
                        pj = ps.tile([HD, QTILE], F32, name=f"pj_{wname}", tag="pj")
                        for cb in range(CB):
                            nc.tensor.matmul(
                                pj[:],
                                wr[:, cb, :],
                                xt_t[:, cb, :],
                                start=(cb == 0),
                                stop=(cb == CB - 1),
                            )
                        if "norope" in ablate:
                            nc.vector.tensor_copy(dst[:, t0 : t0 + QTILE], pj[:])
                            continue
                        pjs = npool.tile([HD, QTILE], F32R, name="pjs", tag="pjs")
                        nc.vector.tensor_mul(pjs[:], pj[:], sin_t[:, t0 : t0 + QTILE])
                        pjc = npool.tile([HD, QTILE], F32, name="pjc", tag="pjc")
                        nc.vector.tensor_mul(pjc[:], pj[:], cos_t[:, t0 : t0 + QTILE])
                        rope_ps = ps.tile([HD, QTILE], F32, name="rope_ps", tag="st")
                        nc.tensor.matmul(rope_ps[:], psw_r[:], pjs[:], start=True, stop=True)
                        nc.vector.tensor_add(dst[:, t0 : t0 + QTILE], rope_ps[:], pjc[:])

                    # v projection -> natural layout with ones column
                    pj = ps.tile([HD, QTILE], F32, name="pj_v", tag="pj")
                    for cb in range(CB):
                        nc.tensor.matmul(
                            pj[:],
                            wv_r[:, cb, :],
                            xt_t[:, cb, :],
                            start=(cb == 0),
                            stop=(cb == CB - 1),
                        )
                    vt_sb = vtp.tile([HD, QTILE], F32, name="vt_sb")
                    nc.vector.tensor_copy(vt_sb[:], pj[:])
                    if "novtr" in ablate:
                        continue
                    for kvt in range(4):
                        kv = tt * 4 + kvt
                        c128 = slice(kvt * 128, (kvt + 1) * 128)
                        vtr = ps.tile([128, 128], F32, name="vtr", tag="tr")
                        nc.tensor.transpose(vtr[:], vt_sb[:, c128], ident[:])
                        nc.vector.tensor_copy(va_a[:, kv, 0:D], vtr[:, 0:64])
                        nc.vector.tensor_copy(va_b[:, kv, 0:D], vtr[:, 64:128])
                        nc.vector.tensor_copy(va_a[:, kv, D : D + 1], ones_f[:])
                        nc.vector.tensor_copy(va_b[:, kv, D : D + 1], ones_f[:])

                # ==================== stage 2: attention ====================
                if "noattn" in ablate:
                    # dump qt_b so stage 1 work is observable, skip rest
                    nc.sync.dma_start(out=out_d.ap()[b, 0:HD, 0:C], in_=qt_b.bitcast(F32)[:, 0:C])
                    continue
                for j in range(NJ):
                    q0 = j * QTILE
                    for h in range(HL):
                        hp = h * D
                        va = va_a if h == 0 else va_b
                        yt_ps = ps.tile([D + 1, QTILE], F32, name="yt_ps", tag="yt")
                        nblk = 4 * (j + 1)
                        for k in range(nblk):
                            m = k - 4 * j
                            c0 = 0 if m < 0 else min(m, 2) * 128
                            e0 = 0 if m < 0 else m * 128
                            st = ps.tile([128, QTILE], F32, name="st", tag="st")
                            nc.tensor.matmul(
                                st[:, c0:QTILE],
                                kt_b[hp : hp + D, k * 128 : (k + 1) * 128],
                                qt_b[hp : hp + D, q0 + c0 : q0 + QTILE],
                                start=True,
                                stop=True,
                            )
                            pt = ptp.tile([128, QTILE], F32R, name="pt")
                            nc.scalar.activation(
                                pt[:, e0:QTILE], st[:, e0:QTILE], EXP, scale=0.125
                            )
                            if m >= 0:
                                nc.vector.tensor_mul(
                                    pt[:, e0 : e0 + 128], pt[:, e0 : e0 + 128], tri[:]
                                )
                                if e0 > c0:
                                    nc.vector.tensor_copy(
                                        pt[:, c0:e0], zeros[:, 0 : e0 - c0]
                                    )
                            nc.tensor.matmul(
                                yt_ps[:, c0:QTILE],
                                va[:, k, :],
                                pt[:, c0:QTILE],
                                start=(k == 0),
                                stop=(k == nblk - 1),
                            )
                        rc = npool.tile([1, QTILE], F32, name="rc", tag="rc", bufs=3)
                        nc.vector.reciprocal(rc[:], yt_ps[D : D + 1, :])
                        rbc = npool.tile([D, QTILE], F32, name="rbc", tag="rbc", bufs=3)
                        nc.gpsimd.partition_broadcast(rbc[:], rc[:])
                        nc.vector.tensor_mul(
                            yt_b[hp : hp + D, q0 : q0 + QTILE], yt_ps[0:D, :], rbc[:]
                        )

                # ================= stage 3: output projection ================
                if "nos3" in ablate:
                    nc.sync.dma_start(out=out_d.ap()[b, 0:HD, 0:C], in_=yt_b.bitcast(F32)[:, 0:C])
                    continue
                for tb in range(T // 128):
                    for co in range(C // QTILE):
                        op = ps.tile([128, QTILE], F32, name="op", tag="pj")
                        nc.tensor.matmul(
                            op[:],
                            yt_b[:, tb * 128 : (tb + 1) * 128],
                            wo_r[:, co * QTILE : (co + 1) * QTILE],
                            start=True,
                            stop=True,
                        )
                        ot = opool.tile([128, QTILE], F32, name="ot")
                        nc.vector.tensor_copy(ot[:], op[:])
                        nc.sync.dma_start(
                            out=out_d.ap()[
                                b,
                                tb * 128 : (tb + 1) * 128,
                                co * QTILE : (co + 1) * QTILE,
                            ],
                            in_=ot[:],
                        )
    nc.finalize()
    return nc


def _rope_tables():
    freqs = 1.0 / (10000.0 ** (np.arange(0, D, 2, dtype=np.float64) / D))  # [32]
    grid = np.arange(T, dtype=np.float64)[:, None] * freqs[None, :]  # [T, 32]
    cos = np.cos(grid)  # [T, 32]
    sin = np.sin(grid)
    # row d of [D, T] tables uses freq d//2; sin sign: + for even d, - for odd
    cos_b = np.repeat(cos.T, 2, axis=0)  # [64, T]
    sin_b = np.repeat(sin.T, 2, axis=0)
    sin_b[1::2] *= -1.0
    cos_hd = np.tile(cos_b, (HL, 1)).astype(np.float32)  # [128, T]
    sin_hd = np.tile(sin_b, (HL, 1)).astype(np.float32)
    return np.ascontiguousarray(cos_hd), np.ascontiguousarray(sin_hd)


def _pswap():
    p = np.zeros((128, 128), dtype=np.float32)
    idx = np.arange(0, 128, 2)
    p[idx, idx + 1] = 1.0
    p[idx + 1, idx] = 1.0
    return p


def kernel(x, wq, wk, wv, wo):
    if "nc" not in _CACHE:
        _CACHE["nc"] = build()
    nc = _CACHE["nc"]

    cos_hd, sin_hd = _rope_tables()
    psw = _pswap()
    x = np.ascontiguousarray(x, dtype=np.float32)
    core_ids = list(range(NCORES))
    in_maps = []
    for c in core_ids:
        r0 = c * HD
        in_maps.append(
            {
                "x": x,
                "wqt": np.ascontiguousarray(wq[r0 : r0 + HD, :].T),
                "wkt": np.ascontiguousarray(wk[r0 : r0 + HD, :].T),
                "wvt": np.ascontiguousarray(wv[r0 : r0 + HD, :].T),
                "wot": np.ascontiguousarray(wo[:, r0 : r0 + HD].T),
                "cosb": cos_hd,
                "sinb": sin_hd,
                "pswap": psw,
            }
        )
    res = run_bass_kernel_spmd(nc, in_maps, core_ids).results
    out = np.zeros((B, T, C), dtype=np.float32)
    for c in core_ids:
        out += res[c]["out"]
    return out
